# revision 1
# baseline (speedup 1.0000x reference)
"""ConvNMF loss kernel for Trainium2, sharded over 8 NeuronCores.

Math (see reference):
    W = softplus(W_pre)            # (F, K, L)
    H = softplus(H_pre)            # (K, T + L - 1)
    pred[f, t] = sum_{k,l} W[f,k,l] * H[k, T-1+l-t]
    out = sum((pred - data)^2) / (F*T)

Sharding: timebins split across 8 cores (sequence parallel), W replicated,
each core's H shard carries an L-1 halo. Per-core partial SSE is returned as
a [128,1] per-partition vector; the host does the final (tiny) reduction.

Device-side formulation: with Hs[k,j] = softplus(H_pre[k, T+30-t0-j]) (the
host flips each H shard in time), the core computes the forward correlation
    pred[f, t0+i] = sum_{k,l'} W[f,k,31-l'] * Hs[k, i+l']
as 16 accumulating 128-contraction matmuls per output tile: lag pairs
(2j, 2j+1) are packed into the 128 partitions (64 components x 2 lags). The
packed rhs Hdup[0:64,m]=Hs[:,m], Hdup[64:128,m]=Hs[:,m+1] is built host-side
from H_pre (pure reindexing; softplus runs on device).
"""
import numpy as np
from contextlib import ExitStack

import concourse.bass as bass
import concourse.bacc as bacc
import concourse.mybir as mybir
import concourse.tile as tile
from concourse import bass_utils

F32 = mybir.dt.float32
F32R = mybir.dt.float32r
AFT = mybir.ActivationFunctionType

F = 256           # n_features
K = 64            # n_components
L = 32            # n_lags
T = 20000         # n_timebins
NCORES = 8
TC = T // NCORES  # timebins per core (2500)
TW = 500          # output tile width (<=512 fp32 PSUM bank, >=256 for f32r rate)
NT = TC // TW     # 5 tiles per f-chunk
NJ = L // 2       # 16 lag pairs -> 16 matmuls per output tile
FCH = F // 128    # 2 f-chunks of 128
HC = TC + L - 1   # 2531 H columns per core (halo included)
NGROUPS = FCH * NT  # 10 PSUM tiles


FP8 = mybir.dt.float8e4
HPAD = 2544      # hq8 row stride, multiple of 16 (DoubleRow AP constraint)
NQ = L // 4      # 8 quad-lag DoubleRow matmuls per output tile


def build_nc(mode="fp8", reps=1):
    nc = bacc.Bacc("TRN2", target_bir_lowering=False, debug=False)
    hdup_d = nc.dram_tensor("hdup", [128, HC], F32, kind="ExternalInput").ap()
    wmat_d = nc.dram_tensor("wmat", [128, NJ * F], F32, kind="ExternalInput").ap()
    data_d = nc.dram_tensor("dat", [F, TC], F32, kind="ExternalInput").ap()
    out_d = nc.dram_tensor("out", [128, NGROUPS], F32, kind="ExternalOutput").ap()

    mmdt = F32R if mode == "f32r" else F32

    with tile.TileContext(nc) as tc, ExitStack() as ctx:
        cpool = ctx.enter_context(tc.tile_pool(name="cpool", bufs=1))
        dpool = ctx.enter_context(tc.tile_pool(name="dpool", bufs=NGROUPS))
        spool = ctx.enter_context(tc.tile_pool(name="spool", bufs=NGROUPS))
        ppool = ctx.enter_context(tc.tile_pool(name="ppool", bufs=4, space="PSUM"))

        # ---- H: chunked DMA -> exp -> ln(x+1) so the first matmuls can
        # start as soon as the first column chunk of softplus(H) lands ----
        HCH = 640
        hraw = cpool.tile([128, HC], F32, tag="hraw")
        hexp = cpool.tile([128, HC], F32, tag="hexp")
        hsp = cpool.tile([128, HC], mmdt, tag="hsp")
        h_edges = list(range(0, HC, HCH)) + [HC]
        # W is consumed interleaved with the first output tile's matmuls;
        # chunk it the same way (4 chunks of 4 lag pairs).
        WCH = 4
        wchunks = []
        for wc in range(NJ // WCH):
            wraw = cpool.tile([128, WCH * F], F32, tag=f"wraw{wc}",
                              name=f"wraw{wc}")
            wexp = cpool.tile([128, WCH * F], F32, tag=f"wexp{wc}",
                              name=f"wexp{wc}")
            w_t = cpool.tile([128, WCH * F], mmdt, tag=f"wsp{wc}",
                             name=f"wsp{wc}")
            wchunks.append((wraw, wexp, w_t))

        # interleave: H chunk 0, W chunk 0, H chunk 1, W chunks 1-3, rest of H
        def emit_h_chunk(i):
            lo, hi = h_edges[i], h_edges[i + 1]
            nc.sync.dma_start(hraw[:, lo:hi], hdup_d[:, lo:hi])
            nc.scalar.activation(hexp[:, lo:hi], hraw[:, lo:hi], AFT.Exp)
            nc.scalar.activation(hsp[:, lo:hi], hexp[:, lo:hi], AFT.Ln,
                                 bias=1.0)

        def emit_w_chunk(wc):
            wraw, wexp, w_t = wchunks[wc]
            nc.sync.dma_start(wraw[:], wmat_d[:, wc * WCH * F:(wc + 1) * WCH * F])
            nc.scalar.activation(wexp[:], wraw[:], AFT.Exp)
            nc.scalar.activation(w_t[:], wexp[:], AFT.Ln, bias=1.0)

        # fp8 path: DoubleRow matmuls contract 256 rows (2 subtiles); H and
        # W are converted f32->fp8e4 on DVE right after each softplus chunk.
        if mode == "fp8":
            hq8 = cpool.tile([128, 2, HPAD], FP8, tag="hq8")
            wq8 = [cpool.tile([128, WCH * F], FP8, tag=f"wq8{wc}",
                              name=f"wq8{wc}")
                   for wc in range(NJ // WCH)]

        def emit_h8_chunk(i):
            lo, hi = h_edges[i], h_edges[i + 1]
            nc.vector.tensor_copy(hq8[:, 0, lo:hi], hsp[:, lo:hi])
            m0 = max(0, lo - 2)
            nc.vector.tensor_copy(hq8[:, 1, m0:hi - 2], hsp[:, m0 + 2:hi])

        def emit_w8_chunk(wc):
            nc.vector.tensor_copy(wq8[wc][:], wchunks[wc][2][:])

        def post_h(i):
            if mode == "fp8":
                emit_h8_chunk(i)

        def post_w(wc):
            if mode == "fp8":
                emit_w8_chunk(wc)

        emit_h_chunk(0); post_h(0)
        emit_w_chunk(0); post_w(0)
        emit_h_chunk(1); post_h(1)
        emit_w_chunk(1); post_w(1)
        emit_w_chunk(2); post_w(2)
        emit_w_chunk(3); post_w(3)
        for i in range(2, len(h_edges) - 1):
            emit_h_chunk(i); post_h(i)

        def w_lhsT(j, c):
            w_t = wchunks[j // WCH][2]
            off = (j % WCH) * F + c * 128
            return w_t[:, off:off + 128]

        # ---- main loop: 10 output tiles ----
        apool = ctx.enter_context(tc.tile_pool(name="apool", bufs=2))
        acc0 = None
        for r in range(reps):
            acc = apool.tile([128, NGROUPS], F32, tag="acc", name=f"acc{r}")
            if r == 0:
                acc0 = acc
            g = 0
            for c in range(FCH):
                for i0 in range(NT):
                    pt = ppool.tile([128, TW], F32, tag="pt", name=f"pt{r}_{g}")
                    base = i0 * TW
                    if mode == "fp8":
                        for jq in range(NQ):
                            wc, q = jq // 2, jq % 2
                            lhsT = wq8[wc][:, q * 512:(q + 1) * 512].rearrange(
                                "p (i m) -> p i m", i=2)[:, :, c * 128:(c + 1) * 128]
                            rhs = hq8[:, :, base + 4 * jq: base + 4 * jq + TW]
                            nc.tensor.matmul(
                                pt[:], lhsT, rhs, start=(jq == 0),
                                stop=(jq == NQ - 1),
                                perf_mode=mybir.MatmulPerfMode.DoubleRow)
                    else:
                        for j in range(NJ):
                            rhs = hsp[:, base + 2 * j: base + 2 * j + TW]
                            nc.tensor.matmul(pt[:], w_lhsT(j, c), rhs,
                                             start=(j == 0), stop=(j == NJ - 1))

                    dt_ = dpool.tile([128, TW], F32, tag="dt", name=f"dt{r}_{g}")
                    nc.sync.dma_start(
                        dt_[:], data_d[c * 128:(c + 1) * 128, base:base + TW])
                    resid = spool.tile([128, TW], F32, tag="resid",
                                       name=f"resid{r}_{g}")
                    nc.vector.tensor_sub(resid[:], pt[:], dt_[:])
                    sq = spool.tile([128, TW], F32, tag="sq", name=f"sq{r}_{g}")
                    nc.scalar.activation(sq[:], resid[:], AFT.Square,
                                         accum_out=acc[:, g:g + 1])
                    g += 1

        # ---- final: DMA the per-tile column sums out; host sums them ----
        nc.sync.dma_start(out_d[:], acc0[:])
    nc.compile()
    return nc


def make_in_maps(data, W_pre, H_pre):
    """Pure host-side resharding/reindexing (no math beyond indexing)."""
    data = np.ascontiguousarray(data, dtype=np.float32)
    W_pre = np.asarray(W_pre, dtype=np.float32)
    H_pre = np.asarray(H_pre, dtype=np.float32)

    # W: [128, 16*256]; rows (l2*64+k), col block j holds lag pair (2j, 2j+1)
    # wmat[l2*64+k, j*256+f] = W_pre[f, k, 31-(2j+l2)]
    wt = np.transpose(W_pre, (2, 1, 0))          # [L, K, F], wt[l,k,f]
    wmat = np.empty((128, NJ * F), dtype=np.float32)
    for j in range(NJ):
        for l2 in range(2):
            wmat[l2 * K:(l2 + 1) * K, j * F:(j + 1) * F] = wt[31 - (2 * j + l2)]

    in_maps = []
    for c in range(NCORES):
        t0 = c * TC
        # Hs_pre[k, j] = H_pre[k, T+30-t0-j], j in [0, HC)
        hrs = H_pre[:, T - TC - t0: T + L - 1 - t0][:, ::-1]  # [K, HC]
        hdup = np.empty((128, HC), dtype=np.float32)
        hdup[:K] = hrs
        hdup[K:, :HC - 1] = hrs[:, 1:]
        hdup[K:, HC - 1] = hrs[:, HC - 1]  # pad col, never read by matmuls
        in_maps.append({
            "hdup": np.ascontiguousarray(hdup),
            "wmat": wmat,
            "dat": np.ascontiguousarray(data[:, t0:t0 + TC]),
        })
    return in_maps


_CACHED_NC = {}


def run_cores(data, W_pre, H_pre, mode="fp8", trace=False):
    if mode not in _CACHED_NC:
        _CACHED_NC[mode] = build_nc(mode)
    nc = _CACHED_NC[mode]
    in_maps = make_in_maps(data, W_pre, H_pre)
    res = bass_utils.run_bass_kernel_spmd(
        nc, in_maps, core_ids=list(range(NCORES)), trace=trace)
    return res


def kernel(data, W_pre, H_pre):
    res = run_cores(data, W_pre, H_pre, mode="f32r", trace=False)
    sse = np.float64(0.0)
    for r in res.results:
        sse += r["out"].astype(np.float64).sum()
    return np.float32(sse / (F * T))

